# revision 17
# baseline (speedup 1.0000x reference)
"""Multihead attention (custom freq-bias) Trainium2 Bass kernel, v2.

Full inputs -> shard across 8 NeuronCores -> SPMD bass kernel -> host combine.
Core c handles batch b = c//2 and head-half s = c%2 (8 of 16 heads).

Math folds (all exact):
  - K-bias dropped: score' = q.(k+bk) = q.k + q.bk, constant over key axis ->
    softmax invariant.
  - V-bias folded into host bp: att rows sum to 1, so att@(v+bv) = att@v + bv,
    and (y+bv)@Wp + bp/2 = y@Wp + (bv@Wp + bp/2).
  - freq bias folded multiplicatively into v_aug: exp(s+f) = exp(s)*exp(f);
    exp(f_k) scales row k of v_aug (and its ones column), cancels in softmax
    normalization.
  - Q-bias + 1/sqrt(D) scale folded into Wq/bq on host; bq applied at qT
    eviction as a per-partition activation bias.

Per-core structure (fp16 data path, f32 PSUM):
  P1: qT (pair-packed [128ch, 2048q]), kTz (per-head zero-padded stationary
      [128, 2048k], rows of the sibling head zeroed), v_aug (freq-scaled,
      ones column, [128k, 8*65]).
  P2: per (pair, q-half) block, fused kt loop: scores matmul (full 128-mode,
      zero-padded stationary kills sibling head), ScalarE Exp (fp16 out),
      AV matmul with ones column (M=65) accumulating y^T and denominators.
      TensorE ~1706ns/kt == ScalarE ~1706ns/kt -> balanced pipeline.
      Normalization fused into eviction: reciprocal + partition_broadcast +
      tensor_mul(psA, recip) -> yT.
  P3: out_partial = yT @ Wp + bp_eff (DVE eviction adds broadcast bias),
      fp16 out; host sums the two head-half partials.
"""

import numpy as np

import concourse.bass as bass
import concourse.tile as tile
from concourse import bacc, mybir
from concourse.bass import ts

F32 = mybir.dt.float32
F16 = mybir.dt.float16
AF = mybir.ActivationFunctionType

B, N, C, H, D = 4, 2048, 1024, 16, 64
NCORES = 8
HC = C // 2          # 512 channels per core (8 heads x 64)
NH = HC // D         # 8 heads per core
NKT = N // 128       # 16 key tiles
VA_W = NH * (D + 1)  # v_aug width: 8 heads x 65 cols


# --- custom DVE 2-pass exp: exp(s) ~= Q3(v)^8 * R3(v), v = s*DELTA/8 in PSUM ---
DELTA = 0.4861997184703159
QCONST = (1.1215030841032672, 2.390806792398718, 2.343041562891338)
RCONST = (0.4019831184681635, -0.2532289696181654, 0.5285325226573622)
EXP_ACT_SCALE = 8.0 / DELTA          # ACT exp recovers exp(s) via free affine
WQ_EXTRA = DELTA / 8.0               # extra factor on wq/bq beyond 1/sqrt(D)
DVE_KTS = (4, 9, 14)                 # kt indices whose exp runs on DVE

_EXP_OPS = {}


def _register_dve_exp():
    if _EXP_OPS:
        return
    import numpy as _np
    from concourse.dve_spec import (Spec, Src0, Src1, C0, C1, C2, sq, lower,
                                    _has_src1)
    from concourse import dve_ops as _dops
    from concourse.dve_uop import DveOpSpec

    def mk(name, body, ref):
        if name in _dops._SUB_OPCODE_FOR_NAME:
            return next(o for o in _dops.OPS if o.name == name)
        spec = Spec(body=body, reference=ref)
        opcode = max(_dops._SUB_OPCODE_FOR_NAME.values()) + 1
        shas = {}
        for ver in ("v3", "v4"):
            shas[ver] = DveOpSpec(name=name, opcode=opcode,
                                  uops=lower(spec, ver=ver),
                                  rd1_en=_has_src1(spec)).sha(ver)
        op = _dops.DveOp(name, spec, subdim=False, uops_sha=shas)
        _dops.OPS.append(op)
        _dops._SUB_OPCODE_FOR_NAME[name] = opcode
        _dops.CUSTOM_DVE_SPECS[name] = spec
        return op

    _EXP_OPS["q"] = mk(
        "EXP8Q_ANT",
        sq(sq((C0 + C1 * Src0) + sq(Src0) * (C2 + Src0))),
        lambda in0, in1, s0, s1, imm2:
            ((((s0 + s1 * in0) + in0 * in0 * (imm2 + in0)) ** 2) ** 2
             ).astype(_np.float32))
    _EXP_OPS["r"] = mk(
        "EXP8R_ANT",
        sq(Src0) * ((C0 + C1 * Src1) + sq(Src1) * (C2 + Src1)),
        lambda in0, in1, s0, s1, imm2:
            (in0 * in0 * ((s0 + s1 * in1) + in1 * in1 * (imm2 + in1))
             ).astype(_np.float32))


def kernel_body(ctx, tc, out, ins):
    """Emit the per-core kernel. out: [2048, 1024] f16 DRAM. ins: dict of DRAM APs."""
    _register_dve_exp()
    nc = tc.nc
    xq, xk, xv = ins["xqt"], ins["xkt"], ins["xvt"]      # [1024, 2048] f16
    wq, wk, wv = ins["wq"], ins["wk"], ins["wv"]          # [1024, 512] f16
    wp = ins["wp"]                                        # [512, 1024] f16
    bqc = ins["bqc"]                                      # [128, 4] f32
    bpb = ins["bpb"]                                      # [128, 1024] f32
    fexp = ins["fexp"]                                    # [128, 16] f32

    singles = ctx.enter_context(tc.tile_pool(name="singles", bufs=1))

    # ---- persistent SBUF residents ----
    qT = [singles.tile([128, N], F16, name=f"qT{m}") for m in range(4)]
    kTz = [singles.tile([128, N], F16, name=f"kTz{h}") for h in range(NH)]
    yT = [singles.tile([128, N], F16, name=f"yT{m}") for m in range(4)]
    vaug = [singles.tile([128, VA_W], F16, name=f"vaug{i}") for i in range(NKT)]
    wp_sb = [singles.tile([128, C], F16, name=f"wp{i}") for i in range(4)]
    bpb_sb = singles.tile([128, C], F32, name="bpb_sb")
    bqc_sb = singles.tile([128, 4], F32, name="bqc_sb")
    fexp_sb = singles.tile([128, NKT], F32, name="fexp_sb")

    nc.sync.dma_start(out=bpb_sb, in_=bpb)
    nc.sync.dma_start(out=bqc_sb, in_=bqc)
    nc.sync.dma_start(out=fexp_sb, in_=fexp)
    for i in range(4):
        nc.sync.dma_start(out=wp_sb[i], in_=wp[ts(i, 128), :])

    # zero the sibling-head half of each stationary kT tile (once)
    for h in range(NH):
        lo, hi = (64, 128) if h % 2 == 0 else (0, 64)
        nc.vector.memset(kTz[h][lo:hi, :], 0.0)
    # v_aug ones columns: memset 1.0 then scale by exp(freq bias) per kt
    for i in range(NKT):
        ones_col = vaug[i].rearrange("p (h c) -> p h c", c=D + 1)[:, :, D]
        nc.vector.memset(ones_col, 1.0)
        nc.vector.tensor_scalar_mul(ones_col, ones_col, fexp_sb[:, i:i + 1])

    # ================= Phase 1: projections =================
    with (
        tc.tile_pool(name="xpool", bufs=16) as xpool,
        tc.tile_pool(name="wpool", bufs=24) as wpool,
        tc.tile_pool(name="psum1", bufs=4, space="PSUM") as psum1,
    ):
        # preload all projection weights up front
        w_all = {}
        for wname, w_dram in (("q", wq), ("k", wk), ("v", wv)):
            tiles = []
            for k in range(8):
                t = wpool.tile([128, 512], F16, tag="w", name=f"w{wname}{k}")
                nc.sync.dma_start(out=t, in_=w_dram[ts(k, 128), :])
                tiles.append(t)
            w_all[wname] = tiles

        # --- qT: pair-packed [ch, q]; kTz: per-head zero-padded stationary ---
        for which, (wname, x_dram) in enumerate((("q", xq), ("k", xk))):
            w_sb = w_all[wname]
            for nq in range(4):
                x_sb = []
                for k in range(8):
                    t = xpool.tile([128, 512], F16, tag="x", name=f"x{k}")
                    nc.sync.dma_start(out=t, in_=x_dram[ts(k, 128), ts(nq, 512)])
                    x_sb.append(t)
                for m in range(4):
                    ps = psum1.tile([128, 512], F32, tag="ps1", name="ps_qk")
                    for k in range(8):
                        nc.tensor.matmul(ps, w_sb[k][:, ts(m, 128)], x_sb[k],
                                         start=(k == 0), stop=(k == 7))
                    if which == 0:
                        nc.vector.tensor_scalar_add(qT[m][:, ts(nq, 512)], ps,
                                                    bqc_sb[:, m:m + 1])
                    else:
                        nc.scalar.activation(out=kTz[2 * m][0:64, ts(nq, 512)],
                                             in_=ps[0:64, :], func=AF.Identity,
                                             bias=0.0, scale=1.0)
                        nc.scalar.activation(out=kTz[2 * m + 1][64:128, ts(nq, 512)],
                                             in_=ps[64:128, :], func=AF.Identity,
                                             bias=0.0, scale=1.0)

        # --- v_aug: out[n, ch] = x @ Wv, rows scaled by exp(freq bias) ---
        wv_sb = w_all["v"]
        for ntg in range(4):
            xv_sb = []
            for k in range(8):
                t = xpool.tile([128, 512], F16, tag="x", name=f"xv{k}")
                nc.sync.dma_start(out=t, in_=xv[ts(k, 128), ts(ntg, 512)])
                xv_sb.append(t)
            for ntl in range(4):
                nt = ntg * 4 + ntl
                ps = psum1.tile([128, 512], F32, tag="ps1", name="ps_v")
                for k in range(8):
                    nc.tensor.matmul(ps, xv_sb[k][:, ts(ntl, 128)], wv_sb[k],
                                     start=(k == 0), stop=(k == 7))
                va = vaug[nt].rearrange("p (h c) -> p h c", c=D + 1)
                nc.vector.tensor_scalar_mul(
                    va[:, :, 0:D], ps.rearrange("p (h c) -> p h c", c=D),
                    fexp_sb[:, nt:nt + 1])

    # ================= Phase 2: attention (fused kt pipeline) =================
    with (
        tc.tile_pool(name="epool", bufs=8) as epool,
        tc.tile_pool(name="ypool", bufs=10) as ypool,
        tc.tile_pool(name="rpool", bufs=8) as rpool,
        tc.tile_pool(name="tqpool", bufs=3) as tqpool,
        tc.tile_pool(name="bpool", bufs=6) as bpool,
        tc.tile_pool(name="psS", bufs=2, space="PSUM") as psS,
        tc.tile_pool(name="psA", bufs=4, space="PSUM") as psA_pool,
    ):
        for pair in range(4):
            for qh in range(2):
                q0 = qh * 1024
                psA = [psA_pool.tile([128, 512], F32, tag="psA",
                                     name=f"psA{i}") for i in range(4)]
                prev = None

                def issue_av(kt, etA, etB):
                    for idx, et in enumerate((etA, etB)):
                        h = 2 * pair + idx
                        for j in range(2):
                            nc.tensor.matmul(
                                psA[2 * idx + j][0:D + 1, :],
                                vaug[kt][:, h * (D + 1):(h + 1) * (D + 1)],
                                et[:, ts(j, 512)],
                                start=(kt == 0), stop=(kt == NKT - 1),
                                skip_group_check=True)

                for kt in range(NKT):
                    pssA = psS.tile([128, 1024], F32, tag="pss", name="pssA")
                    pssB = psS.tile([128, 1024], F32, tag="pss", name="pssB")
                    for j in range(2):
                        nc.tensor.matmul(pssA[:, ts(j, 512)],
                                         kTz[2 * pair][:, ts(kt, 128)],
                                         qT[pair][:, q0 + j * 512:q0 + (j + 1) * 512],
                                         start=True, stop=True)
                        nc.tensor.matmul(pssB[:, ts(j, 512)],
                                         kTz[2 * pair + 1][:, ts(kt, 128)],
                                         qT[pair][:, q0 + j * 512:q0 + (j + 1) * 512],
                                         start=True, stop=True)
                    etA = epool.tile([128, 1024], F16, tag="et", name="etA")
                    etB = epool.tile([128, 1024], F16, tag="et", name="etB")
                    if kt in DVE_KTS:
                        for et, pss in ((etA, pssA), (etB, pssB)):
                            tq = tqpool.tile([128, 1024], F32, tag="tq", name="tq")
                            nc.vector._custom_dve(
                                _EXP_OPS["q"], out=tq, in0=pss,
                                s0=QCONST[0], s1=QCONST[1], imm2=QCONST[2])
                            nc.vector._custom_dve(
                                _EXP_OPS["r"], out=et, in0=tq, in1=pss,
                                s0=RCONST[0], s1=RCONST[1], imm2=RCONST[2])
                    else:
                        nc.scalar.activation(out=etA, in_=pssA, func=AF.Exp,
                                             bias=0.0, scale=EXP_ACT_SCALE)
                        nc.scalar.activation(out=etB, in_=pssB, func=AF.Exp,
                                             bias=0.0, scale=EXP_ACT_SCALE)
                    if prev is not None:
                        issue_av(*prev)
                    prev = (kt, etA, etB)
                issue_av(*prev)

                # fast PSUM eviction (frees psA for the next block), then
                # async normalization: yT = yu[0:D] * (1 / den), den = yu[D]
                for idx in range(2):
                    po = idx * 64
                    for j in range(2):
                        pa = psA[2 * idx + j]
                        yu = ypool.tile([128, 512], F32, tag="yu", name="yu")
                        nc.vector.tensor_copy(yu[0:D, :], pa[0:D, :])
                        dr = rpool.tile([1, 512], F32, tag="dr", name="dr")
                        nc.vector.tensor_copy(dr, pa[D:D + 1, :])
                        dbc = bpool.tile([64, 512], F32, tag="dbc", name="dbc")
                        nc.gpsimd.partition_broadcast(dbc, dr)
                        rec = bpool.tile([64, 512], F32, tag="rec", name="rec")
                        nc.vector.reciprocal_approx_fast(out=rec, in_=dbc)
                        nc.gpsimd.tensor_mul(
                            yT[pair][po:po + 64, q0 + j * 512:q0 + (j + 1) * 512],
                            yu[0:D, :], rec)

    # ================= Phase 3: output projection =================
    with (
        tc.tile_pool(name="opool", bufs=4) as opool,
        tc.tile_pool(name="psO", bufs=6, space="PSUM") as psO,
    ):
        for m in range(16):
            for n2 in range(2):
                ps = psO.tile([128, 512], F32, tag="psO", name="psO")
                for kp in range(4):
                    nc.tensor.matmul(ps, yT[kp][:, ts(m, 128)],
                                     wp_sb[kp][:, ts(n2, 512)],
                                     start=(kp == 0), stop=(kp == 3))
                ot = opool.tile([128, 512], F16, tag="ot", name="ot")
                nc.vector.tensor_add(ot, ps, bpb_sb[:, ts(n2, 512)])
                nc.sync.dma_start(out=out[ts(m, 128), ts(n2, 512)], in_=ot)


INPUT_SPECS = {
    "xqt": ([C, N], F16), "xkt": ([C, N], F16), "xvt": ([C, N], F16),
    "wq": ([C, HC], F16), "wk": ([C, HC], F16), "wv": ([C, HC], F16),
    "wp": ([HC, C], F16),
    "bqc": ([128, 4], F32),
    "bpb": ([128, C], F32),
    "fexp": ([128, NKT], F32),
}


def build_nc():
    from contextlib import ExitStack
    _register_dve_exp()
    nc = bacc.Bacc("TRN2", target_bir_lowering=False, debug=False)
    ins = {name: nc.dram_tensor(name, shape, dt, kind="ExternalInput").ap()
           for name, (shape, dt) in INPUT_SPECS.items()}
    out = nc.dram_tensor("out", [N, C], F16, kind="ExternalOutput").ap()
    with tile.TileContext(nc) as tc:
        with ExitStack() as ctx:
            kernel_body(ctx, tc, out, ins)
    nc.compile()
    return nc


def make_fexp():
    fr = np.linspace(0.0, 1.0, N, dtype=np.float32)
    fb = -((fr - 0.5) ** 2) * 10.0
    return np.ascontiguousarray(np.exp(fb).reshape(NKT, 128).T).astype(np.float32)


def make_shards(inputs):
    """Full inputs -> list of 8 per-core input dicts."""
    q = np.asarray(inputs["query"], np.float32)
    k = np.asarray(inputs["key"], np.float32)
    v = np.asarray(inputs["value"], np.float32)
    Wq = np.asarray(inputs["Wq"], np.float32); bq = np.asarray(inputs["bq"], np.float32)
    Wk = np.asarray(inputs["Wk"], np.float32)
    Wv = np.asarray(inputs["Wv"], np.float32); bv = np.asarray(inputs["bv"], np.float32)
    Wp = np.asarray(inputs["Wp"], np.float32); bp = np.asarray(inputs["bp"], np.float32)
    fexp = make_fexp()
    scale = np.float32(1.0 / np.sqrt(D))

    shards = []
    for c in range(NCORES):
        b, s = c // 2, c % 2
        cs = slice(s * HC, (s + 1) * HC)
        bp_eff = 0.5 * bp + bv[cs] @ Wp[cs, :]
        sh = {
            "xqt": np.ascontiguousarray(q[b].T).astype(np.float16),
            "xkt": np.ascontiguousarray(k[b].T).astype(np.float16),
            "xvt": np.ascontiguousarray(v[b].T).astype(np.float16),
            "wq": (np.ascontiguousarray(Wq[:, cs]) * (scale * WQ_EXTRA)
                   ).astype(np.float16),
            "wk": np.ascontiguousarray(Wk[:, cs]).astype(np.float16),
            "wv": np.ascontiguousarray(Wv[:, cs]).astype(np.float16),
            "wp": np.ascontiguousarray(Wp[cs, :]).astype(np.float16),
            "bqc": np.ascontiguousarray(
                (bq[cs] * scale * WQ_EXTRA).reshape(4, 128).T).astype(np.float32),
            "bpb": np.tile(bp_eff.astype(np.float32), (128, 1)),
            "fexp": fexp,
        }
        shards.append(sh)
    return shards


_NC_CACHE = None


def kernel(**inputs):
    global _NC_CACHE
    shards = make_shards(inputs)
    if _NC_CACHE is None:
        _NC_CACHE = build_nc()
    nc = _NC_CACHE
    from concourse import bass_utils
    res = bass_utils.run_bass_kernel_spmd(nc, shards, core_ids=list(range(NCORES)))
    outs = [np.asarray(r["out"], np.float32) for r in res.results]
    full = np.stack([outs[2 * b] + outs[2 * b + 1] for b in range(B)])
    return full.astype(np.float32)
